# revision 22
# baseline (speedup 1.0000x reference)
"""Trainium2 Bass kernel for nn_BoundaryLoss_49306224558104.

Math note: in the reference, every pixel is either foreground (where
neg = edt(~fg) is exactly 0) or background (where pos = edt(fg) is
exactly 0), so min(pos, neg) == 0 at every pixel and dist_map is
identically zero (bitwise-exact in f32: the EDT of a pixel whose own
d0 is 0 takes the y==j / k==i branch with cost 0, and sqrt(0) == 0).
The loss therefore reduces exactly to mean(softplus(x) - x*z) with
x = pred.squeeze(1), z = (target > 0).

Sharding: pure data-parallel - sample b goes to core b (B == 8 ==
n_cores). Per core, the sample's pred and target (cast to bf16 on
host; z is 0/1 so bf16 is exact, and bf16 pred costs ~1e-7 relative
on the final mean - tolerance is 2e-2) are packed into one
[128, 1024] bf16 DRAM buffer. Two DMAs on the sync HWDGE ring, pred
first: the ~750 ns DMA_DIRECT2D descriptor-gen latency and ~600 ns
HWDGE first-byte latency are fixed, so halving the bytes (vs f32)
pulls the exp start earlier. softplus(x) = ln(1 + exp(x)) on the
scalar engine (|x| < 5, so the direct form is exact; this build's
act tables have exp+ln in one set but no softplus table). Row sums
come from the activation / scalar_tensor_tensor accumulators; one
ones-vector matmul over both accumulator columns collapses the 128
partition partials to a single [1, 2] PSUM value so the output DMA
is one 8-byte descriptor. The compiler-injected teardown (a fixed
~8.9 us serial reset of the full 256-entry semaphore file, round-
robin across all 5 engines - measured invariant to kernel content)
retires the in-flight output DMA. Host combines the 8 x [1, 2]
partials into the scalar mean.
"""

import numpy as np

B, H, W = 8, 256, 256
P, F = 128, 512  # H*W == P*F
FX2 = 2 * F
N_CORES = 8


def _build_nc():
    import concourse.bass as bass
    import concourse.mybir as mybir

    nc = bass.Bass(trn_type="TRN2")

    # Both inputs are staged TRANSPOSED [F, P] so dma_start_transpose reads
    # 16-row x 128-col xbar tiles = 4 KB contiguous DRAM chunks: 32 + 16 +
    # 16 descriptors total vs 256 for plain per-partition DMAs. The DMA
    # phase measures descriptor-processing-bound (~85 ns/descriptor per
    # SDMA engine, ~3x worse when the part is thermally throttled), so the
    # descriptor count sets both the mean and the variance. target is
    # split in two half transposes so the vector multiply can start on the
    # first half while the second is still in flight.
    xq = nc.declare_dram_parameter("xq", [F, P], mybir.dt.bfloat16, isOutput=False)
    tq = nc.declare_dram_parameter("tq", [F, P], mybir.dt.bfloat16, isOutput=False)
    out = nc.declare_dram_parameter("out", [1, 3], mybir.dt.float32, isOutput=True)

    zeros128 = nc.const_aps.aps[(mybir.dt.float32, 0.0)]  # [128,1] framework const
    ones128 = nc.const_aps.aps[(mybir.dt.float32, 1.0)]  # [128,1] framework const

    with (
        nc.sbuf_tensor("xtt", [P, FX2], mybir.dt.bfloat16) as xtt,
        nc.sbuf_tensor("e", [P, F], mybir.dt.float32) as e,
        nc.sbuf_tensor("l", [P, F], mybir.dt.float32) as l,
        nc.sbuf_tensor("xz", [P, F], mybir.dt.bfloat16) as xz,
        nc.sbuf_tensor("sums", [P, 3], mybir.dt.float32) as sums,
        nc.sbuf_tensor("trash", [P, 1], mybir.dt.float32) as trash,
        nc.sbuf_tensor("res", [1, 3], mybir.dt.float32) as res,
        nc.psum_tensor("ps", [1, 3], mybir.dt.float32) as ps,
        nc.psum_tensor("ps_warm", [1, 2], mybir.dt.float32) as ps_warm,
        nc.semaphore("x_sem") as x_sem,
        nc.semaphore("t_sem") as t_sem,
        nc.semaphore("t2_sem") as t2_sem,
        nc.semaphore("s_sem") as s_sem,
        nc.semaphore("a_sem") as a_sem,
        nc.semaphore("v_sem") as v_sem,
        nc.semaphore("m_sem") as m_sem,
        nc.semaphore("r_sem") as r_sem,
        nc.semaphore("o_sem") as o_sem,
    ):
        x = xtt[:, 0:F]  # pred logits, bf16
        tf = xtt[:, F:FX2]  # target as bf16 0/1

        # Single basic block: walrus assigns activation-table sets per basic
        # block, so one block means one exp+ln table set, loaded once at the
        # ungated dummy activation below - hidden under the input DMA.

        # input DMAs on the sync HWDGE ring, pred first (it gates the ACT
        # chain; target only feeds the off-critical-path vector multiply)
        FH = F // 2
        nc.sync.dma_start_transpose(out=xtt[:, 0:F], in_=xq[:, :]).then_inc(x_sem, 16)
        nc.sync.dma_start_transpose(out=xtt[:, F : F + FH], in_=tq[0:FH, :]).then_inc(
            t_sem, 16
        )
        nc.sync.dma_start_transpose(out=xtt[:, F + FH : FX2], in_=tq[FH:F, :]).then_inc(
            t2_sem, 16
        )

        # scalar engine: dummy activation forces the PWP table load now;
        # then softplus(x) = ln(1 + exp(x)) with a row-sum accumulator
        nc.scalar.activation(trash[:, :], zeros128, mybir.ActivationFunctionType.Exp)
        nc.scalar.wait_ge(x_sem, 16)
        nc.scalar.activation(e[:, :], x, mybir.ActivationFunctionType.Exp)
        # same-engine RAW on `e`: flush the ACT pipeline before Ln reads it
        # (a bare drain() fails walrus codegen; give it a sem update). The
        # queue is in-order, so no wait on s_sem is needed before Ln.
        nc.scalar.drain().then_inc(s_sem, 1)
        nc.scalar.activation(
            l[:, :],
            e[:, :],
            mybir.ActivationFunctionType.Ln,
            bias=1.0,
            accum_out=sums[:, 0:1],
        ).then_inc(a_sem, 1)

        # vector engine: xz = (x * 1.0) * tf ; sums[:,1:3] = row-sums(xz),
        # one half per target transfer so the first multiply starts while
        # the second half is still in flight.
        # (tensor_tensor_reduce is broken in this walrus build - "ISA wrong
        # length" - scalar_tensor_tensor+accum is the working equivalent.)
        nc.vector.wait_ge(x_sem, 16)
        nc.vector.wait_ge(t_sem, 16)
        nc.vector.scalar_tensor_tensor(
            out=xz[:, 0:FH],
            in0=x[:, 0:FH],
            scalar=1.0,
            in1=tf[:, 0:FH],
            op0=mybir.AluOpType.mult,
            op1=mybir.AluOpType.mult,
            accum_out=sums[:, 1:2],
        ).then_inc(v_sem, 1)
        nc.vector.wait_ge(t2_sem, 16)
        nc.vector.scalar_tensor_tensor(
            out=xz[:, FH:F],
            in0=x[:, FH:F],
            scalar=1.0,
            in1=tf[:, FH:F],
            op0=mybir.AluOpType.mult,
            op1=mybir.AluOpType.mult,
            accum_out=sums[:, 2:3],
        ).then_inc(v_sem, 1)

        # tensor engine: warm-up matmul under the DMA shadow, then one
        # matmul collapsing all accumulator columns (softplus + 2x xz) at
        # once - a_sem (the ACT chain) is the later gate
        nc.tensor.matmul(ps_warm[:, 0:1], ones128, ones128, start=True, stop=True)
        nc.tensor.wait_ge(v_sem, 2)
        nc.tensor.wait_ge(a_sem, 1)
        nc.tensor.matmul(
            ps[:, 0:3], ones128, sums[:, 0:3], start=True, stop=True
        ).then_inc(m_sem, 1)

        # bounce the matmul result PSUM -> SBUF (DMA can't read PSUM)
        nc.vector.wait_ge(m_sem, 1)
        nc.vector.tensor_copy(res[:, :], ps[:, :]).then_inc(r_sem, 1)

        # output DMA: one 8-byte descriptor with its (mandatory) completion
        # semaphore, but no completion wait and no explicit end barrier -
        # the compiler-injected teardown retires the in-flight 8-byte
        # write long before the NEFF ends
        nc.sync.wait_ge(r_sem, 1)
        nc.sync.dma_start(out=out[:, :], in_=res[:, :], single_packet=True).then_inc(
            o_sem, 16
        )

    return nc


def _pack(pred: np.ndarray, target: np.ndarray) -> tuple[np.ndarray, np.ndarray]:
    import ml_dtypes

    xq = np.ascontiguousarray(pred.reshape(B, P, F).transpose(0, 2, 1)).astype(
        ml_dtypes.bfloat16
    )
    tq = (target.reshape(B, P, F).transpose(0, 2, 1) > 0).astype(ml_dtypes.bfloat16)
    return xq, np.ascontiguousarray(tq)


def kernel(pred: np.ndarray, target: np.ndarray) -> np.ndarray:
    from concourse.bass_utils import run_bass_kernel_spmd

    pred = np.asarray(pred, dtype=np.float32)
    target = np.asarray(target)

    xq, tq = _pack(pred, target)

    nc = _build_nc()
    in_maps = [{"xq": xq[b], "tq": tq[b]} for b in range(B)]
    res = run_bass_kernel_spmd(nc, in_maps, list(range(N_CORES)))

    total = 0.0
    for r in res.results:
        o = r["out"].astype(np.float64)
        total += o[0, 0] - o[0, 1] - o[0, 2]
    return np.array(total / (B * H * W), dtype=np.float32)


# revision 30
# speedup vs baseline: 1.0936x; 1.0936x over previous
"""Trainium2 Bass kernel for nn_BoundaryLoss_49306224558104.

Math note: in the reference, every pixel is either foreground (where
neg = edt(~fg) is exactly 0) or background (where pos = edt(fg) is
exactly 0), so min(pos, neg) == 0 at every pixel and dist_map is
identically zero (bitwise-exact in f32: the EDT of a pixel whose own
d0 is 0 takes the y==j / k==i branch with cost 0, and sqrt(0) == 0).
The loss therefore reduces exactly to mean(softplus(x) - x*z) with
x = pred.squeeze(1), z = (target > 0).

Sharding: pure data-parallel - sample b goes to core b (B == 8 ==
n_cores). Per core, the sample's pred and target (cast to bf16 on
host; z is 0/1 so bf16 is exact, and bf16 pred costs ~1e-7 relative
on the final mean - tolerance is 2e-2) are packed into one
[128, 1024] bf16 DRAM buffer. Two DMAs on the sync HWDGE ring, pred
first: the ~750 ns DMA_DIRECT2D descriptor-gen latency and ~600 ns
HWDGE first-byte latency are fixed, so halving the bytes (vs f32)
pulls the exp start earlier. softplus(x) = ln(1 + exp(x)) on the
scalar engine (|x| < 5, so the direct form is exact; this build's
act tables have exp+ln in one set but no softplus table). Row sums
come from the activation / scalar_tensor_tensor accumulators; one
ones-vector matmul over both accumulator columns collapses the 128
partition partials to a single [1, 2] PSUM value so the output DMA
is one 8-byte descriptor. The compiler-injected teardown (a fixed
~8.9 us serial reset of the full 256-entry semaphore file, round-
robin across all 5 engines - measured invariant to kernel content)
retires the in-flight output DMA. Host combines the 8 x [1, 2]
partials into the scalar mean.
"""

import numpy as np

B, H, W = 8, 256, 256
P, F = 128, 512  # H*W == P*F
FX2 = 2 * F
N_CORES = 8


def _build_nc():
    import concourse.bass as bass
    import concourse.mybir as mybir

    nc = bass.Bass(trn_type="TRN2")

    # Both inputs are staged TRANSPOSED [F, P] so dma_start_transpose reads
    # 16-row x 128-col xbar tiles = 4 KB contiguous DRAM chunks: 32 + 32
    # descriptors total vs 256 for plain per-partition DMAs. The DMA phase
    # measures descriptor-processing-bound (~85 ns/descriptor per SDMA
    # engine, ~3x worse when the part is thermally throttled), so the
    # descriptor count sets both the mean and the variance. Exactly two
    # transposes: the DMA_TRANSPOSE instruction costs a fixed ~1.25 us of
    # queue occupancy regardless of size, so finer splits arrive later.
    xq = nc.declare_dram_parameter("xq", [F, P], mybir.dt.bfloat16, isOutput=False)
    tq = nc.declare_dram_parameter("tq", [F, P], mybir.dt.bfloat16, isOutput=False)
    out = nc.declare_dram_parameter("out", [1, 2], mybir.dt.float32, isOutput=True)

    zeros128 = nc.const_aps.aps[(mybir.dt.float32, 0.0)]  # [128,1] framework const
    ones128 = nc.const_aps.aps[(mybir.dt.float32, 1.0)]  # [128,1] framework const

    with (
        nc.sbuf_tensor("xtt", [P, FX2], mybir.dt.bfloat16) as xtt,
        nc.sbuf_tensor("e", [P, F], mybir.dt.float32) as e,
        nc.sbuf_tensor("l", [P, F], mybir.dt.float32) as l,
        nc.sbuf_tensor("xz", [P, F], mybir.dt.bfloat16) as xz,
        nc.sbuf_tensor("sums", [P, 2], mybir.dt.float32) as sums,
        nc.sbuf_tensor("trash", [P, 1], mybir.dt.float32) as trash,
        nc.sbuf_tensor("res", [1, 2], mybir.dt.float32) as res,
        nc.psum_tensor("ps", [1, 2], mybir.dt.float32) as ps,
        nc.psum_tensor("ps_warm", [1, 2], mybir.dt.float32) as ps_warm,
        nc.semaphore("x_sem") as x_sem,
        nc.semaphore("t_sem") as t_sem,
        nc.semaphore("s_sem") as s_sem,
        nc.semaphore("a_sem") as a_sem,
        nc.semaphore("v_sem") as v_sem,
        nc.semaphore("m_sem") as m_sem,
        nc.semaphore("r_sem") as r_sem,
        nc.semaphore("o_sem") as o_sem,
    ):
        x = xtt[:, 0:F]  # pred logits, bf16
        tf = xtt[:, F:FX2]  # target as bf16 0/1

        # Single basic block: walrus assigns activation-table sets per basic
        # block, so one block means one exp+ln table set, loaded once at the
        # ungated dummy activation below - hidden under the input DMA.

        # input DMAs on the sync HWDGE ring, pred first (it gates the ACT
        # chain; target only feeds the off-critical-path vector multiply)
        nc.sync.dma_start_transpose(out=xtt[:, 0:F], in_=xq[:, :]).then_inc(x_sem, 16)
        nc.sync.dma_start_transpose(out=xtt[:, F:FX2], in_=tq[:, :]).then_inc(
            t_sem, 16
        )

        # scalar engine: dummy activation forces the PWP table load now;
        # then softplus(x) = ln(1 + exp(x)) with a row-sum accumulator
        nc.scalar.activation(trash[:, :], zeros128, mybir.ActivationFunctionType.Exp)
        nc.scalar.wait_ge(x_sem, 16)
        nc.scalar.activation(e[:, :], x, mybir.ActivationFunctionType.Exp)
        # same-engine RAW on `e`: flush the ACT pipeline before Ln reads it
        # (a bare drain() fails walrus codegen; give it a sem update). The
        # queue is in-order, so no wait on s_sem is needed before Ln.
        nc.scalar.drain().then_inc(s_sem, 1)
        nc.scalar.activation(
            l[:, :],
            e[:, :],
            mybir.ActivationFunctionType.Ln,
            bias=1.0,
            accum_out=sums[:, 0:1],
        ).then_inc(a_sem, 1)

        # vector engine: xz = (x * 1.0) * tf ; sums[:,1] = row-sum(xz)
        # (tensor_tensor_reduce is broken in this walrus build - "ISA wrong
        # length" - scalar_tensor_tensor+accum is the working equivalent.)
        nc.vector.wait_ge(x_sem, 16)
        nc.vector.wait_ge(t_sem, 16)
        nc.vector.scalar_tensor_tensor(
            out=xz[:, :],
            in0=x,
            scalar=1.0,
            in1=tf,
            op0=mybir.AluOpType.mult,
            op1=mybir.AluOpType.mult,
            accum_out=sums[:, 1:2],
        ).then_inc(v_sem, 1)

        # tensor engine: warm-up matmul under the DMA shadow, then one
        # matmul collapsing both accumulator columns (softplus + xz) at
        # once
        nc.tensor.matmul(ps_warm[:, 0:1], ones128, ones128, start=True, stop=True)
        nc.tensor.wait_ge(v_sem, 1)
        nc.tensor.wait_ge(a_sem, 1)
        nc.tensor.matmul(
            ps[:, 0:2], ones128, sums[:, 0:2], start=True, stop=True
        ).then_inc(m_sem, 1)

        # bounce the matmul result PSUM -> SBUF (DMA can't read PSUM)
        nc.vector.wait_ge(m_sem, 1)
        nc.vector.tensor_copy(res[:, :], ps[:, :]).then_inc(r_sem, 1)

        # output DMA: one 8-byte descriptor with its (mandatory) completion
        # semaphore, but no completion wait and no explicit end barrier -
        # the compiler-injected teardown retires the in-flight 8-byte
        # write long before the NEFF ends
        nc.sync.wait_ge(r_sem, 1)
        nc.sync.dma_start(out=out[:, :], in_=res[:, :], single_packet=True).then_inc(
            o_sem, 16
        )

    return nc


def _pack(pred: np.ndarray, target: np.ndarray) -> tuple[np.ndarray, np.ndarray]:
    import ml_dtypes

    xq = np.ascontiguousarray(pred.reshape(B, P, F).transpose(0, 2, 1)).astype(
        ml_dtypes.bfloat16
    )
    tq = (target.reshape(B, P, F).transpose(0, 2, 1) > 0).astype(ml_dtypes.bfloat16)
    return xq, np.ascontiguousarray(tq)


def kernel(pred: np.ndarray, target: np.ndarray) -> np.ndarray:
    from concourse.bass_utils import run_bass_kernel_spmd

    pred = np.asarray(pred, dtype=np.float32)
    target = np.asarray(target)

    xq, tq = _pack(pred, target)

    nc = _build_nc()
    in_maps = [{"xq": xq[b], "tq": tq[b]} for b in range(B)]
    res = run_bass_kernel_spmd(nc, in_maps, list(range(N_CORES)))

    total = 0.0
    for r in res.results:
        o = r["out"].astype(np.float64)
        total += o[0, 0] - o[0, 1]
    return np.array(total / (B * H * W), dtype=np.float32)
